# revision 6
# baseline (speedup 1.0000x reference)
"""Child-Sum Tree-LSTM (complete binary tree, depth 15, heap layout)
on 8 Trainium2 cores.

Sharding: data-parallel over nodes within each level (heap chunks =
self-contained subtrees per core; core j owns the subtree rooted at
level-3 node 7+bitrev3(j)).  Levels 14..3 run with zero communication;
one AllGather ships the 8 level-3 (h,c) pairs everywhere; levels 2..0
are computed replicated.

Key design points (v2):
  - Butterfly node storage per level ([left-children | right-children]
    blocks, pure host-side permutation): every child access is a packed
    contiguous half -> DVE 2x fp16 mode everywhere, no stride-2 APs.
  - All big-level matmuls fp8 DoubleRow at 0.5 cyc/row; h stored as
    unscaled fp8 (x16 folded into the h-part weights at scale 1024),
    so h-writes are plain tensor_muls that can run on GpSimd (Pool).
  - Big levels: no hsum -- i/o/u psum chains accumulate the two child
    halves directly (PE absorbs it; ACT is the binding engine).
    Per-mo c/h tails in fp16 keep ACT/DVE/Pool pipelined; the last
    mo's h8-write runs on DVE to shorten the level-boundary tail.
  - Small levels (10..3 sharded + replicated top): x-parts for all 262
    nodes precomputed once (gx, fp16, bias folded in); each level's
    gates then need only h-part matmuls plus ONE fp16 identity-matmul
    per psum bank that injects gx straight into the accumulation group
    (no DVE dequant/bias ops on the critical chain).  Levels 10..8 use
    fp8 DoubleRow h (validated ~1.0e-2 rel err vs the 2e-2 gate);
    7..0 stay fp16.
  - AllGather is ncfw-latency-bound (~36us) -- remote_dma alternatives
    fault this runtime (exec-unit unrecoverable even for sem-only
    relative-dest sends), so the collective stays.

Measured (this host, interleaved repeat-delta bench): ~124 us vs
~172 us for the previous staged kernel; rel err 1.04e-2.
"""

import os
import sys

import numpy as np

for _p in ("/opt/trn_rl_repo",):
    if _p not in sys.path and os.path.isdir(_p):
        sys.path.insert(0, _p)

import concourse.bacc as bacc
import concourse.mybir as mybir
import concourse.tile as tile
from concourse.bass_utils import run_bass_kernel_spmd

P = 128
H = 512
D = 512
DEPTH = 15
NCORES = 8
MT = 512
F32 = mybir.dt.float32
F32R = mybir.dt.float32r
F16 = mybir.dt.float16
F8 = mybir.dt.float8e4
DRMODE = mybir.MatmulPerfMode.DoubleRow
MULT = mybir.AluOpType.mult
ADD = mybir.AluOpType.add

WS, XS = 64.0, 16.0                  # fp8 operand scales: W, x
HSW = 16.0                           # folded into h-part W (h unscaled)
DQ = 1.0 / (WS * XS)                 # psum dequant (x and h share 16)

BIG_LVLS = (14, 13, 12, 11)
SMALL_LVLS = list(range(10, 2, -1))  # 10..3 sharded
FP8_SMALL = (10, 9, 8)               # fp8 small levels
M_BIG = {lvl: 2 ** lvl // NCORES for lvl in BIG_LVLS}
M_SM = {lvl: 2 ** lvl // NCORES for lvl in SMALL_LVLS}
XB_OFF = {}
_acc = 0
for _l in BIG_LVLS:
    XB_OFF[_l] = _acc
    _acc += M_BIG[_l]
NBIG = _acc                          # 3840
XS_OFF = {}
_acc = 0
for _l in SMALL_LVLS:
    XS_OFF[_l] = _acc
    _acc += M_SM[_l]
XS_TOP = _acc                        # 255; then 7 top nodes
NSM = _acc + 7                       # 262

SIG = mybir.ActivationFunctionType.Sigmoid
TANH = mybir.ActivationFunctionType.Tanh
IDENT = mybir.ActivationFunctionType.Identity

_CACHE = {}


def _build_nc(repeat=1, bench_dummy=False, sim1=False, stop_after=None,
              serialize=False):
    nc = bacc.Bacc("TRN2", target_bir_lowering=False, debug=False,
                   num_devices=1 if sim1 else NCORES)

    kw = {} if bench_dummy else {"kind": "ExternalInput"}
    xt8 = nc.dram_tensor("xt8", [P, 2, 2, NBIG], F8, **kw)
    xt16 = nc.dram_tensor("xt16", [P, 4, NSM], F16, **kw)
    w8i = nc.dram_tensor("w8i", [P, 4, 2, H], F8, **kw)
    w8o = nc.dram_tensor("w8o", [P, 4, 2, H], F8, **kw)
    w8u = nc.dram_tensor("w8u", [P, 4, 2, H], F8, **kw)
    w8fx = nc.dram_tensor("w8fx", [P, 2, 2, H], F8, **kw)
    w8fh = nc.dram_tensor("w8fh", [P, 2, 2, H], F8, **kw)
    # fp16 x-part weights for the gx precompute (small levels)
    w16i = nc.dram_tensor("w16i", [P, 4, H], F16, **kw)
    w16o = nc.dram_tensor("w16o", [P, 4, H], F16, **kw)
    w16u = nc.dram_tensor("w16u", [P, 4, H], F16, **kw)
    w16fx = nc.dram_tensor("w16fx", [P, 4, H], F16, **kw)
    # fp16 h-part weights for the fp16 small levels
    wih = nc.dram_tensor("wih", [P, 4, H], F16, **kw)
    woh = nc.dram_tensor("woh", [P, 4, H], F16, **kw)
    wuh = nc.dram_tensor("wuh", [P, 4, H], F16, **kw)
    wfh16 = nc.dram_tensor("wfh16", [P, 4, H], F16, **kw)
    bias = nc.dram_tensor("bias", [P, 16], F32, **kw)
    ident = nc.dram_tensor("ident", [P, 2, P], F16, **kw)
    hc_out = nc.dram_tensor("hc_out", [2, 4, P], F32, kind="ExternalOutput")

    with tile.TileContext(nc) as tc:
        with (
            tc.tile_pool(name="wpool", bufs=1) as wpool,
            tc.tile_pool(name="hbuf", bufs=1) as hbuf,
            tc.tile_pool(name="h16", bufs=1) as h16p,
            tc.tile_pool(name="g16", bufs=2) as g16p,
            tc.tile_pool(name="gw", bufs=2) as gwp,
            tc.tile_pool(name="ps2", bufs=1, space="PSUM") as ps2,
            tc.tile_pool(name="ps1", bufs=2, space="PSUM") as ps1,
            tc.tile_pool(name="ps0", bufs=1, space="PSUM") as ps0,
            tc.tile_pool(name="dram", bufs=1, space="DRAM") as dram,
        ):
            bias_s = wpool.tile([P, 16], F32, tag="bias")
            nc.sync.dma_start(bias_s[:], bias[:])
            id_s = wpool.tile([P, 2, P], F16, tag="ident", name="id_s")
            nc.sync.dma_start(id_s[:], ident[:])
            idK = id_s[:, 0]     # 1024*I (fp8-level gx injection)
            idK1 = id_s[:, 1]    # I (fp16-level gx injection)

            x8_s = wpool.tile([P, 2, 2, NBIG], F8, tag="x8", name="x8_s")
            x16_s = wpool.tile([P, 4, NSM], F16, tag="x16", name="x16_s")

            _XCHUNKS = ((0, 1024), (1024, 2048), (2048, 3072), (3072, NBIG))

            def load_x(first):
                for c0, c1 in (_XCHUNKS[:1] if first else _XCHUNKS[1:]):
                    nc.sync.dma_start(x8_s[:, :, :, c0:c1],
                                      xt8[:, :, :, c0:c1])
                if not first:
                    nc.sync.dma_start(x16_s[:], xt16[:])

            _x0_loaded = [False]

            def load_w8():
                """Cold-start order: x-part rows (kd 0,1) of i/o/u and
                the first two 512-col x chunks go first — exactly what
                the leaf's first matmul chains consume."""
                ws = {}
                for nm, t, kdn in (("i", w8i, 4), ("o", w8o, 4),
                                   ("u", w8u, 4), ("fx", w8fx, 2),
                                   ("fh", w8fh, 2)):
                    ws[nm] = wpool.tile([P, kdn, 2, H], F8,
                                        tag=f"w8{nm}", name=f"w8{nm}_s")
                first = not _x0_loaded[0]
                if first:
                    _x0_loaded[0] = True
                    nc.scalar.dma_start(x8_s[:, :, :, 0:512],
                                        xt8[:, :, :, 0:512])
                    nc.sync.dma_start(ws["i"][:, 0], w8i[:, 0])
                    nc.sync.dma_start(ws["i"][:, 1], w8i[:, 1])
                    nc.scalar.dma_start(x8_s[:, :, :, 512:1024],
                                        xt8[:, :, :, 512:1024])
                    nc.sync.dma_start(ws["o"][:, 0], w8o[:, 0])
                    nc.sync.dma_start(ws["o"][:, 1], w8o[:, 1])
                    nc.sync.dma_start(ws["u"][:, 0], w8u[:, 0])
                    nc.sync.dma_start(ws["u"][:, 1], w8u[:, 1])
                    for nm, t in (("i", w8i), ("o", w8o), ("u", w8u)):
                        for kd in (2, 3):
                            nc.sync.dma_start(ws[nm][:, kd], t[:, kd])
                    for nm, t in (("fx", w8fx), ("fh", w8fh)):
                        for kd in (0, 1):
                            nc.sync.dma_start(ws[nm][:, kd], t[:, kd])
                else:
                    for nm, t, kdn in (("i", w8i, 4), ("o", w8o, 4),
                                       ("u", w8u, 4), ("fx", w8fx, 2),
                                       ("fh", w8fh, 2)):
                        for kd in range(kdn):
                            nc.sync.dma_start(ws[nm][:, kd], t[:, kd])
                return ws

            def load_w16():
                ws = {}
                for nm, t in (("i", w16i), ("o", w16o), ("u", w16u),
                              ("fx", w16fx), ("ih", wih), ("oh", woh),
                              ("uh", wuh), ("fh", wfh16)):
                    s = wpool.tile([P, 4, H], F16, tag=f"w16{nm}",
                                   name=f"w16{nm}_s")
                    for ko in range(4):
                        nc.sync.dma_start(s[:, ko], t[:, ko])
                    ws[nm] = s
                return ws

            # level state buffers (butterfly storage order)
            h8A = hbuf.tile([P, 4, 1024], F8, tag="h8A")
            h8B = hbuf.tile([P, 4, 1024], F8, tag="h8B")
            cA = hbuf.tile([P, 4, 1024], F16, tag="cA")
            cB = hbuf.tile([P, 4, 1024], F16, tag="cB")
            h8S = hbuf.tile([P, 4, 256], F8, tag="h8S")    # small fp8 ping
            h8T = hbuf.tile([P, 4, 256], F8, tag="h8T")
            cA2 = hbuf.tile([P, 4, 256], F16, tag="cA2")
            cB2 = hbuf.tile([P, 4, 256], F16, tag="cB2")
            hA = h16p.tile([P, 4, 64], F16, tag="hA", name="hA")
            hB = h16p.tile([P, 4, 64], F16, tag="hB", name="hB")
            h3g = hbuf.tile([P, 4, 8], F32, tag="h3g")
            c3g = hbuf.tile([P, 4, 8], F32, tag="c3g")
            h3g16 = h16p.tile([P, 4, 8], F16, tag="h3g16", name="h3g16")
            c3g16 = h16p.tile([P, 4, 8], F16, tag="c3g16", name="c3g16")
            gxf2 = hbuf.tile([P, 4, 2 * NSM], F16, tag="gxf2",
                             name="gxf2")
            gx = {}
            for g in ("i", "o", "u", "f"):
                gx[g] = hbuf.tile([P, 4, NSM], F16, tag=f"gx{g}",
                                  name=f"gx_{g}")

            def psum_g(tag):
                """2KB psum (one bank), [P, MT] f32."""
                return ps2.tile([P, MT], F32, tag=tag, name=f"ps_{tag}")

            def psum_pair():
                """4KB psum (two banks), [P, 2, MT] f32."""
                return ps1.tile([P, 2, MT], F32, tag="fl", name="ps_pair")

            def leaf_half(w8, xc0, out_c, out_h8):
                """1024 leaf cols in 2 subtiles sharing stationaries."""
                oc0 = xc0 % 1024
                for mo in range(4):
                    ms = slice(mo * P, (mo + 1) * P)
                    gt = {}
                    for g, bcol in (("i", 0), ("o", 4), ("u", 8)):
                        w = w8[g]
                        g2 = g16p.tile([P, 2, MT], F16, tag=f"l{g}",
                                       name=f"l16_{g}")
                        gt[g] = g2
                        pj = psum_pair()
                        for j in (0, 1):
                            c0 = xc0 + j * MT
                            for kd in range(2):
                                nc.tensor.matmul(
                                    pj[:, j], w[:, kd, :, ms],
                                    x8_s[:, kd, :, c0:c0 + MT],
                                    start=(kd == 0), stop=(kd == 1),
                                    perf_mode=DRMODE)
                        fn = TANH if g == "u" else SIG
                        nc.scalar.activation(
                            g2[:], pj[:], fn, scale=DQ,
                            bias=bias_s[:, bcol + mo:bcol + mo + 1])
                    c_sl = out_c[:, mo, oc0:oc0 + 1024]
                    nc.vector.tensor_mul(
                        c_sl,
                        gt["i"].rearrange("p a b -> p (a b)"),
                        gt["u"].rearrange("p a b -> p (a b)"))
                    tt2 = g16p.tile([P, 2, MT], F16, tag="ltt",
                                    name="tt2L")
                    nc.scalar.activation(
                        tt2.rearrange("p a b -> p (a b)"), c_sl, TANH)
                    nc.gpsimd.tensor_mul(
                        out_h8[:, mo, oc0:oc0 + 1024],
                        tt2.rearrange("p a b -> p (a b)"),
                        gt["o"].rearrange("p a b -> p (a b)"))

            def big_internal(w8, xc0, m, ch_h8, ch_c, out_c, oc0,
                             out_h8, h8oc0):
                """Internal fp8 level subtile (m<=512 cols), butterfly
                children at ch_h8/ch_c cols [0:m | m:2m]."""
                for mo in range(4):
                    ms = slice(mo * P, (mo + 1) * P)
                    g16 = {}
                    for g, bcol in (("i", 0), ("o", 4), ("u", 8)):
                        p = psum_g(g)[:, :m]
                        w = w8[g]
                        for kd in range(2):
                            nc.tensor.matmul(
                                p[:], w[:, kd, :, ms],
                                x8_s[:, kd, :, xc0:xc0 + m],
                                start=(kd == 0), stop=False,
                                perf_mode=DRMODE)
                        for kd in range(2):
                            for hf in (0, 1):
                                nc.tensor.matmul(
                                    p[:], w[:, 2 + kd, :, ms],
                                    ch_h8[:, 2 * kd:2 * kd + 2,
                                          hf * m:(hf + 1) * m],
                                    start=False,
                                    stop=(kd == 1 and hf == 1),
                                    perf_mode=DRMODE)
                        dst = g16p.tile([P, MT], F16, tag=f"b{g}",
                                        name=f"b16_{g}")[:, :m]
                        g16[g] = dst
                        fn = TANH if g == "u" else SIG
                        nc.scalar.activation(
                            dst[:], p[:], fn, scale=DQ,
                            bias=bias_s[:, bcol + mo:bcol + mo + 1])
                    pf = psum_pair()[:, :, :m]
                    for kd in range(2):
                        w = w8["fx"][:, kd, :, ms]
                        xsl = x8_s[:, kd, :, xc0:xc0 + m]
                        nc.tensor.matmul(pf[:, 0], w, xsl, start=(kd == 0),
                                         stop=False, perf_mode=DRMODE)
                        nc.tensor.matmul(pf[:, 1], w, xsl, start=(kd == 0),
                                         stop=False, perf_mode=DRMODE)
                    for kd in range(2):
                        w = w8["fh"][:, kd, :, ms]
                        nc.tensor.matmul(
                            pf[:, 0], w, ch_h8[:, 2 * kd:2 * kd + 2, 0:m],
                            start=False, stop=(kd == 1), perf_mode=DRMODE)
                        nc.tensor.matmul(
                            pf[:, 1], w, ch_h8[:, 2 * kd:2 * kd + 2, m:2 * m],
                            start=False, stop=(kd == 1), perf_mode=DRMODE)
                    f16 = g16p.tile([P, 2, MT], F16, tag="bf",
                                    name="b16_f")[:, :, :m]
                    nc.scalar.activation(f16[:], pf[:], SIG, scale=DQ,
                                         bias=bias_s[:, 12 + mo:13 + mo])
                    # per-mo c/h tail (fp16, butterfly-packed)
                    c_sl = out_c[:, mo, oc0:oc0 + m]
                    nc.vector.tensor_mul(c_sl, g16["i"][:], g16["u"][:])
                    fc = g16p.tile([P, 2, MT], F16, tag="bfc",
                                   name="fcb")[:, :, :m]
                    nc.vector.tensor_mul(
                        fc[:],
                        f16[:],
                        ch_c[:, mo, 0:2 * m].rearrange(
                            "p (t m) -> p t m", t=2))
                    nc.vector.tensor_add(c_sl, c_sl, fc[:, 0])
                    nc.vector.tensor_add(c_sl, c_sl, fc[:, 1])
                    tt = g16p.tile([P, MT], F16, tag="btt",
                                   name="ttb")[:, :m]
                    nc.scalar.activation(tt[:], c_sl, TANH)
                    eng = nc.vector if mo == 3 else nc.gpsimd
                    eng.tensor_mul(
                        out_h8[:, mo, h8oc0:h8oc0 + m], tt[:],
                        g16["o"][:])

            def precompute_gx(w16):
                """Batched fp16 x-parts (+bias on Pool) for small+top."""
                for mo in range(4):
                    ms = slice(mo * P, (mo + 1) * P)
                    for gi, (g, wnm, bcol) in enumerate(
                            (("i", "i", 0), ("o", "o", 4),
                             ("u", "u", 8), ("f", "fx", 12))):
                        w_s = w16[wnm]
                        ps = ps0.tile([P, MT], F32, tag="pre",
                                      name="ps_pre")[:, :NSM]
                        for ko in range(4):
                            nc.tensor.matmul(
                                ps[:], w_s[:, ko, ms], x16_s[:, ko],
                                start=(ko == 0), stop=(ko == 3))
                        c0 = bcol + mo
                        nc.vector.tensor_add(
                            gx[g][:, mo], ps[:],
                            bias_s[:, c0:c0 + 1].to_broadcast((P, NSM)))
                blocks = [(XS_OFF[lvl], M_SM[lvl]) for lvl in SMALL_LVLS]
                blocks += [(XS_TOP + 3, 4), (XS_TOP + 1, 2), (XS_TOP, 1)]
                for off, ml in blocks:
                    for hf in (0, 1):
                        nc.vector.tensor_copy(
                            gxf2[:, :, 2 * off + hf * ml:
                                 2 * off + (hf + 1) * ml],
                            gx["f"][:, :, off:off + ml])

            def cpath(m2, i16, o16, u16, f16, ch_c, out_c, out_h,
                      out_h8=None):
                """Shared fp16 c/h tail for small levels.  i/o/u16
                [P,4,m] views, f16 [P,4,2m] halves-packed."""
                c_blk = out_c[:, :, 0:m2]
                nc.vector.tensor_mul(c_blk, i16, u16)
                fc = g16p.tile([P, 4, 2 * P], F16, tag="fcs",
                               name="fcs")[:, :, :2 * m2]
                nc.vector.tensor_mul(fc[:], f16, ch_c[:, :, 0:2 * m2])
                nc.vector.tensor_add(c_blk, c_blk, fc[:, :, 0:m2])
                nc.vector.tensor_add(c_blk, c_blk, fc[:, :, m2:2 * m2])
                tt = g16p.tile([P, 4, P], F16, tag="tts",
                               name="tts")[:, :, :m2]
                nc.scalar.activation(tt[:], c_blk, TANH)
                if out_h is not None:
                    nc.vector.tensor_mul(out_h[:, :, 0:m2], o16, tt[:])
                if out_h8 is not None:
                    nc.vector.tensor_mul(out_h8[:, :, 0:m2], o16, tt[:])

            def small8(w8, m, gxoff, ch_h8, ch_c, out_c, out_h8,
                       out_h16=None):
                """fp8 small level (m<=128): DR matmuls off child fp8 h;
                gx (incl bias) injected into psum via 1024*I fp16 MM."""
                gsl = slice(gxoff, gxoff + m)
                psf_f = psum_pair().rearrange("p t mt -> p (t mt)")
                psf = psf_f[:, :8 * m].rearrange(
                    "p (a m2) -> p a m2", a=4)
                g2 = gxf2[:, :, 2 * gxoff:2 * gxoff + 2 * m]
                if 8 * m <= MT:
                    nc.tensor.matmul(psf_f[:, 0:8 * m], idK[:], g2,
                                     start=True, stop=False)
                else:
                    nc.tensor.matmul(psf_f[:, 0:MT], idK[:], g2[:, 0:2],
                                     start=True, stop=False)
                    nc.tensor.matmul(psf_f[:, MT:2 * MT], idK[:],
                                     g2[:, 2:4], start=True, stop=False)
                ps_g = {}
                for g in ("i", "o", "u"):
                    pg_f = psum_g(g)
                    nc.tensor.matmul(pg_f[:, 0:4 * m], idK[:],
                                     gx[g][:, :, gsl], start=True,
                                     stop=False)
                    ps_g[g] = pg_f[:, :4 * m].rearrange(
                        "p (a m) -> p a m", a=4)
                bank1 = 3 if 8 * m <= MT else 1
                for mo in range(4):
                    ms = slice(mo * P, (mo + 1) * P)
                    for kd in range(2):
                        nc.tensor.matmul(
                            psf[:, mo], w8["fh"][:, kd, :, ms],
                            ch_h8[:, 2 * kd:2 * kd + 2, 0:2 * m],
                            start=False,
                            stop=(kd == 1 and mo in (bank1, 3)),
                            perf_mode=DRMODE)
                    for g in ("i", "o", "u"):
                        for kd in range(2):
                            for hf in (0, 1):
                                nc.tensor.matmul(
                                    ps_g[g][:, mo],
                                    w8[g][:, 2 + kd, :, ms],
                                    ch_h8[:, 2 * kd:2 * kd + 2,
                                          hf * m:(hf + 1) * m],
                                    start=False,
                                    stop=(kd == 1 and hf == 1
                                          and mo == 3),
                                    perf_mode=DRMODE)
                f16 = g16p.tile([P, 4, 2 * P], F16, tag="fss",
                                name="f16s")[:, :, :2 * m]
                i16 = g16p.tile([P, 4, P], F16, tag="is",
                                name="i16s")[:, :, :m]
                o16 = g16p.tile([P, 4, P], F16, tag="os",
                                name="o16s")[:, :, :m]
                u16 = g16p.tile([P, 4, P], F16, tag="us",
                                name="u16s")[:, :, :m]
                nc.scalar.activation(i16[:], ps_g["i"][:], SIG, scale=DQ)
                nc.scalar.activation(u16[:], ps_g["u"][:], TANH, scale=DQ)
                nc.scalar.activation(f16[:], psf[:], SIG, scale=DQ)
                nc.scalar.activation(o16[:], ps_g["o"][:], SIG, scale=DQ)
                cpath(m, i16[:], o16[:], u16[:], f16[:], ch_c, out_c,
                      out_h16, out_h8=out_h8)
            def small16(w16, m, gxoff, ch_h, ch_c, out_c, out_h,
                        ch_h8=None):
                """fp16 small level; ch_h8 given -> cast children first.
                gx (incl bias) injected into psum via I fp16 MM."""
                m2 = m
                gsl = slice(gxoff, gxoff + m2)
                if ch_h8 is not None:
                    hch = h16p.tile([P, 4, 2 * P], F16, tag="hch",
                                    name="hch")[:, :, :2 * m2]
                    nc.vector.tensor_copy(hch[:], ch_h8[:, :, 0:2 * m2])
                else:
                    hch = ch_h[:, :, 0:2 * m2]
                hs = h16p.tile([P, 4, P], F16, tag="hs16",
                               name="hs")[:, :, :m2]
                nc.vector.tensor_add(hs[:], hch[:, :, 0:m2],
                                     hch[:, :, m2:2 * m2])
                psf_f = psum_pair().rearrange("p t mt -> p (t mt)")
                psf = psf_f[:, :8 * m2].rearrange(
                    "p (a m2) -> p a m2", a=4)
                g2 = gxf2[:, :, 2 * gxoff:2 * gxoff + 2 * m2]
                nc.tensor.matmul(psf_f[:, 0:8 * m2], idK1[:], g2,
                                 start=True, stop=False)
                ps_g = {}
                for g in ("i", "o", "u"):
                    pg_f = psum_g(g)
                    nc.tensor.matmul(pg_f[:, 0:4 * m2], idK1[:],
                                     gx[g][:, :, gsl], start=True,
                                     stop=False)
                    ps_g[g] = pg_f[:, :4 * m2].rearrange(
                        "p (a m) -> p a m", a=4)
                for mo in range(4):
                    ms = slice(mo * P, (mo + 1) * P)
                    for ko in range(4):
                        nc.tensor.matmul(
                            psf[:, mo], w16["fh"][:, ko, ms],
                            hch[:, ko], start=False,
                            stop=(ko == 3 and mo == 3))
                    for g, wnm in (("i", "ih"), ("o", "oh"), ("u", "uh")):
                        for ko in range(4):
                            nc.tensor.matmul(
                                ps_g[g][:, mo], w16[wnm][:, ko, ms],
                                hs[:, ko], start=False,
                                stop=(ko == 3 and mo == 3))
                f16 = g16p.tile([P, 4, 2 * P], F16, tag="fss",
                                name="f16s")[:, :, :2 * m2]
                i16 = g16p.tile([P, 4, P], F16, tag="is",
                                name="i16s")[:, :, :m2]
                o16 = g16p.tile([P, 4, P], F16, tag="os",
                                name="o16s")[:, :, :m2]
                u16 = g16p.tile([P, 4, P], F16, tag="us",
                                name="u16s")[:, :, :m2]
                nc.scalar.activation(i16[:], ps_g["i"][:], SIG)
                nc.scalar.activation(u16[:], ps_g["u"][:], TANH)
                nc.scalar.activation(f16[:], psf[:], SIG)
                nc.scalar.activation(o16[:], ps_g["o"][:], SIG)
                cpath(m2, i16[:], o16[:], u16[:], f16[:], ch_c, out_c,
                      out_h)

            if repeat == 0:
                nc.sync.dma_start(
                    hc_out[:],
                    xt8.bitcast(F32)[0:2].rearrange(
                        "a kd i n -> a (kd i) n")[:, :, 0:P])
            _x_rest_loaded = [False]
            for _rep in range(repeat):
                if serialize and _rep > 0:
                    nc.vector.scalar_tensor_tensor(
                        x8_s[:, 0, 0, 0:NBIG:512], hA[:, 0, 0:8], 0.0,
                        x8_s[:, 0, 0, 0:NBIG:512], MULT, ADD)
                w8 = load_w8()
                if not _x_rest_loaded[0]:
                    load_x(first=False)
                    _x_rest_loaded[0] = True
                w16 = load_w16()
                with nc.named_scope("L14h0"):
                    leaf_half(w8, 0, cA, h8A)
                with nc.named_scope("pre"):
                    precompute_gx(w16)
                with nc.named_scope("L13j0"):
                    big_internal(w8, XB_OFF[13], 512, h8A, cA, cB, 0,
                                 h8B, 0)
                with nc.named_scope("L14h1"):
                    leaf_half(w8, 1024, cA, h8A)
                with nc.named_scope("L13j1"):
                    big_internal(w8, XB_OFF[13] + 512, 512, h8A, cA, cB,
                                 512, h8B, 512)
                with nc.named_scope("L12"):
                    big_internal(w8, XB_OFF[12], 512, h8B, cB, cA, 0,
                                 h8A, 0)
                with nc.named_scope("L11"):
                    big_internal(w8, XB_OFF[11], 256, h8A, cA, cA2, 0,
                                 h8S, 0)
                if stop_after == "L11":
                    hf11 = h16p.tile([P, 4, 1], F32, tag="hfin",
                                     name="hf11")
                    cf11 = h16p.tile([P, 4, 1], F32, tag="cfin",
                                     name="cf11")
                    nc.vector.tensor_copy(hf11[:], h8S[:, :, 0:1])
                    nc.vector.tensor_copy(cf11[:], cA2[:, :, 0:1])
                    nc.sync.dma_start(
                        hc_out[0:1].rearrange("one ko p -> p ko one"),
                        hf11[:])
                    nc.sync.dma_start(
                        hc_out[1:2].rearrange("one ko p -> p ko one"),
                        cf11[:])
                    continue
                # fp8 small levels 10..8: (h8S,cA2) -> ... ping-pong
                cur_h8, cur_c = h8S, cA2
                for lvl in FP8_SMALL:
                    nxt_h8 = h8T if cur_h8 is h8S else h8S
                    nxt_c = cB2 if cur_c is cA2 else cA2
                    m = M_SM[lvl]
                    with nc.named_scope(f"L{lvl}"):
                        small8(w8, m, XS_OFF[lvl], cur_h8, cur_c,
                               nxt_c, nxt_h8,
                               out_h16=(hA if lvl == 8 else None))
                    cur_h8, cur_c = nxt_h8, nxt_c
                # fp16 small levels 7..3
                cur_h = hA
                for lvl in SMALL_LVLS:
                    if lvl > 7:
                        continue
                    nxt_h = hB if cur_h is hA else hA
                    nxt_c = cB2 if cur_c is cA2 else cA2
                    with nc.named_scope(f"L{lvl}"):
                        small16(w16, M_SM[lvl], XS_OFF[lvl], cur_h,
                                cur_c, nxt_c, nxt_h)
                    cur_h, cur_c = nxt_h, nxt_c

                if stop_after == "L3":
                    hf3 = h16p.tile([P, 4, 1], F32, tag="hfin", name="hf3")
                    cf3 = h16p.tile([P, 4, 1], F32, tag="cfin", name="cf3")
                    nc.vector.tensor_copy(hf3[:], cur_h[:, :, 0:1])
                    nc.vector.tensor_copy(cf3[:], cur_c[:, :, 0:1])
                    nc.sync.dma_start(
                        hc_out[0:1].rearrange("one ko p -> p ko one"),
                        hf3[:])
                    nc.sync.dma_start(
                        hc_out[1:2].rearrange("one ko p -> p ko one"),
                        cf3[:])
                    continue
                with nc.named_scope("gather"):
                    cc_in = dram.tile([2, 4, P], F32R, name="cc_in")
                    cc_out = dram.tile([8, 2, 4, P], F32R, name="cc_out")
                    hc3 = h16p.tile([P, 2, 4], F32R, tag="h3f",
                                    name="hc3")
                    nc.vector.tensor_copy(
                        hc3[:, 0], cur_h[:, :, 0])
                    nc.vector.tensor_copy(
                        hc3[:, 1], cur_c[:, :, 0])
                    nc.sync.dma_start(
                        cc_in.rearrange("t ko p -> p t ko"), hc3[:])
                    if sim1:
                        for r in range(NCORES):
                            nc.sync.dma_start(cc_out[r], cc_in[:])
                    else:
                        nc.gpsimd.collective_compute(
                            "AllGather", mybir.AluOpType.bypass,
                            replica_groups=[list(range(NCORES))],
                            ins=[cc_in.opt()], outs=[cc_out.opt()])
                    for ko in range(4):
                        nc.sync.dma_start(
                            h3g[:, ko, 0:8],
                            cc_out.bitcast(F32)[:, 0, ko].rearrange(
                                "r p -> p r"))
                        nc.sync.dma_start(
                            c3g[:, ko, 0:8],
                            cc_out.bitcast(F32)[:, 1, ko].rearrange(
                                "r p -> p r"))
                    nc.vector.tensor_copy(h3g16[:], h3g[:])
                    nc.vector.tensor_copy(c3g16[:], c3g[:])

                with nc.named_scope("L2"):
                    small16(w16, 4, XS_TOP + 3, h3g16, c3g16, cB2, hB)
                with nc.named_scope("L1"):
                    small16(w16, 2, XS_TOP + 1, hB, cB2, cA2, hA)
                with nc.named_scope("L0"):
                    small16(w16, 1, XS_TOP + 0, hA, cA2, cB2, hB)

                hfin = h16p.tile([P, 4, 1], F32, tag="hfin", name="hfin")
                cfin = h16p.tile([P, 4, 1], F32, tag="cfin", name="cfin")
                nc.vector.tensor_copy(hfin[:], hB[:, :, 0:1])
                nc.vector.tensor_copy(cfin[:], cB2[:, :, 0:1])
                # serialize-token source tile (bench): mirror into hA
                if serialize:
                    nc.vector.tensor_copy(hA[:, 0, 0:1], hB[:, 0, 0:1])
                nc.sync.dma_start(
                    hc_out[0:1].rearrange("one ko p -> p ko one"),
                    hfin[:])
                nc.sync.dma_start(
                    hc_out[1:2].rearrange("one ko p -> p ko one"),
                    cfin[:])

    nc.compile()
    return nc


def _bitrev3(j):
    return ((j & 1) << 2) | (j & 2) | ((j & 4) >> 2)


def _btf(par):
    return [2 * v + 1 for v in par] + [2 * v + 2 for v in par]


def _core_orders(j):
    """Butterfly storage order per level for core j."""
    root = 7 + _bitrev3(j)
    A = {3: [root]}
    for lvl in range(4, 14):
        A[lvl] = _btf(A[lvl - 1])
    A[14] = _btf(A[13][:512]) + _btf(A[13][512:])
    return A


def _prep_inputs(x, Wi, bi, Wf, bf, Wo, bo, Wu, bu):
    import ml_dtypes
    E4 = ml_dtypes.float8_e4m3
    Wi, Wf, Wo, Wu = (np.asarray(w, np.float32) for w in (Wi, Wf, Wo, Wu))

    def wt8(wpart, scale=WS):  # [H(M),512(K)] -> [P,2(kd),2,H] fp8
        a = wpart.T.reshape(2, 2, P, H).transpose(2, 0, 1, 3)
        return np.ascontiguousarray(a * scale).astype(E4)

    def wt16(wpart):  # [H, 512] -> [P, 4(ko), H] fp16
        a = wpart.T.reshape(4, P, H).transpose(1, 0, 2)
        return np.ascontiguousarray(a).astype(np.float16)

    w8 = {}
    for nm, w in (("i", Wi), ("o", Wo), ("u", Wu)):
        w8[nm] = np.concatenate(
            [wt8(w[:, :D]), wt8(w[:, D:], WS * HSW)], axis=1)
    w8fx, w8fh = wt8(Wf[:, :D]), wt8(Wf[:, D:], WS * HSW)
    w16 = {nm: wt16(w[:, :D]) for nm, w in
           (("i", Wi), ("o", Wo), ("u", Wu), ("fx", Wf))}
    w16h = {nm: wt16(w[:, D:]) for nm, w in
            (("ih", Wi), ("oh", Wo), ("uh", Wu), ("fh", Wf))}
    bias = np.stack(
        [np.asarray(b, np.float32).reshape(4, P) for b in (bi, bo, bu, bf)],
        axis=0)
    bias = np.ascontiguousarray(bias.reshape(16, P).T).astype(np.float32)

    ident_np = np.stack([np.eye(P, dtype=np.float16) * 1024.0,
                         np.eye(P, dtype=np.float16)], axis=1)
    ident_np = np.ascontiguousarray(ident_np)   # [P, 2, P]
    x = np.asarray(x, dtype=np.float32)
    in_maps = []
    for j in range(NCORES):
        A = _core_orders(j)
        bcols = A[14] + A[13] + A[12] + A[11]
        scols = []
        for lvl in SMALL_LVLS:
            scols.extend(A[lvl])
        scols.extend([0, 1, 2, 3, 5, 4, 6])
        xb = x[bcols]                            # [NBIG, 512]
        x8 = np.ascontiguousarray(
            xb.T.reshape(2, 2, P, NBIG).transpose(2, 0, 1, 3) * XS
        ).astype(E4)
        xsm = x[scols]                           # [NSM, 512]
        x16 = np.ascontiguousarray(
            xsm.T.reshape(4, P, NSM).transpose(1, 0, 2)).astype(np.float16)
        in_maps.append({
            "xt8": x8, "xt16": x16,
            "w8i": w8["i"], "w8o": w8["o"], "w8u": w8["u"],
            "w8fx": w8fx, "w8fh": w8fh,
            "w16i": w16["i"], "w16o": w16["o"], "w16u": w16["u"],
            "w16fx": w16["fx"],
            "wih": w16h["ih"], "woh": w16h["oh"], "wuh": w16h["uh"],
            "wfh16": w16h["fh"], "bias": bias, "ident": ident_np,
        })
    return in_maps


def _make_caller(nc):
    """Non-blocking sharded caller for bench (dummy-input builds)."""
    import jax
    from jax.sharding import Mesh, PartitionSpec
    from jax.experimental.shard_map import shard_map
    from concourse import bass2jax
    from concourse.bass2jax import _bass_exec_p, install_neuronx_cc_hook

    install_neuronx_cc_hook()
    partition_name = (nc.partition_id_tensor.name
                      if nc.partition_id_tensor else None)
    out_names, out_avals, zero_outs = [], [], []
    for alloc in nc.m.functions[0].allocations:
        if not isinstance(alloc, mybir.MemoryLocationSet):
            continue
        if alloc.kind == "ExternalOutput":
            shape = tuple(alloc.tensor_shape)
            dtype = mybir.dt.np(alloc.dtype)
            out_names.append(alloc.memorylocations[0].name)
            out_avals.append(jax.core.ShapedArray(shape, dtype))
            zero_outs.append(np.zeros(shape, dtype))
    full_in_names = list(out_names)
    if partition_name is not None:
        full_in_names.append(partition_name)

    def _body(*args):
        operands = list(args)
        if partition_name is not None:
            operands.append(bass2jax.partition_id_tensor())
        return tuple(_bass_exec_p.bind(
            *operands, out_avals=tuple(out_avals),
            in_names=tuple(full_in_names), out_names=tuple(out_names),
            lowering_input_output_aliases=(), sim_require_finite=True,
            sim_require_nnan=True, nc=nc))

    devices = jax.devices()[:NCORES]
    mesh = Mesh(np.asarray(devices), ("core",))
    n_outs = len(out_avals)
    sharded = jax.jit(
        shard_map(_body, mesh=mesh,
                  in_specs=(PartitionSpec("core"),) * n_outs,
                  out_specs=(PartitionSpec("core"),) * n_outs,
                  check_rep=False),
        donate_argnums=tuple(range(n_outs)), keep_unused=True)

    def call():
        czeros = [np.zeros((NCORES * z.shape[0], *z.shape[1:]), z.dtype)
                  for z in zero_outs]
        return sharded(*czeros)
    return call


def bench(reps=(2, 18), iters=40, stop_after=None, serialize=True,
          batches=8):
    """Async-pipelined, batch-interleaved delta timing."""
    import time
    import jax
    calls = []
    for rep in reps:
        nc = _build_nc(repeat=rep, bench_dummy=True,
                       stop_after=stop_after, serialize=serialize)
        call = _make_caller(nc)
        jax.block_until_ready(call())
        calls.append(call)

    def batch(call):
        t0 = time.perf_counter()
        outs = [call() for _ in range(iters)]
        jax.block_until_ready(outs)
        return (time.perf_counter() - t0) / iters

    batch(calls[0]); batch(calls[1])  # extra warm
    diffs = []
    for k in range(batches):
        a, b = (0, 1) if k % 2 == 0 else (1, 0)
        ta = batch(calls[a])
        tb = batch(calls[b])
        d = (tb - ta) if a == 0 else (ta - tb)
        diffs.append(d)
    diffs.sort()
    n = len(diffs)
    mid = diffs[n // 4: n - n // 4] or diffs
    med = sum(mid) / len(mid)
    print(f"  bench diffs(ms): {[f'{d*1e3:.2f}' for d in diffs]}")
    return med / (reps[1] - reps[0]) * 1e9


def kernel(x, Wi, bi, Wf, bf, Wo, bo, Wu, bu):
    if "nc" not in _CACHE:
        _CACHE["nc"] = _build_nc()
    nc = _CACHE["nc"]
    in_maps = _prep_inputs(x, Wi, bi, Wf, bf, Wo, bo, Wu, bu)
    res = run_bass_kernel_spmd(nc, in_maps, core_ids=list(range(NCORES)))
    out = res.results[0]["hc_out"]               # [2, 4, 128]
    h0 = np.ascontiguousarray(out[0].reshape(H)).astype(np.float32)
    c0 = np.ascontiguousarray(out[1].reshape(H)).astype(np.float32)
    return h0, c0
